# revision 7
# baseline (speedup 1.0000x reference)
import math
import sys

sys.path.insert(0, "/opt/trn_rl_repo")

import numpy as np

N_CORES = 8
B, T, D = 65536, 64, 10
B_CORE = B // N_CORES  # 8192
P128 = 128

_cache = {}


def build(Pv, sharpv, Lv, b_core=B_CORE, tb=16):
    """Build + compile the per-core SPMD Bass program.

    Math: s_t = x_t + y_t, carry c_t (c_0 = 0), u_t = s_t + c_t:
      c_{t+1} = sigmoid(sharp*(u_t - 9.5))
      logits[t,d] = L*cos((2pi/P)*(u_t - d))

    On-chip: h_t = tanh(sharp*(u_t-9.5)/2) = 2*c_{t+1}-1 (Tanh+Sin share an
    ACT table set; Sigmoid+Sin do not), angle tile Z = delta*u + pi/2 with
    delta = 2pi/P:
      Z_t = (delta/2)*h_{t-1} + [delta*(s_t - 9) + gamma]   (h_{-1} = -1)
      logits[t,d]   = L*sin(wrap(Z_t - delta*d))    d in [0,5)
      logits[t,d+5] = -logits[t,d]                  (5*delta = pi for P=10)

    Perf rules (from HW traces): compute-engine WRITES must be unit-stride
    (strided writes run ~14x slow); strided READS are ~free. So every op
    writes contiguous; the one reorder (d-major S -> d-minor L) happens in
    the scatter op via a strided READ view, and DMA out moves 640B runs.
    """
    import concourse.bacc as bacc
    import concourse.mybir as mybir
    import concourse.tile as tile

    fp32 = mybir.dt.float32
    i32 = mybir.dt.int32
    A = mybir.ActivationFunctionType
    Alu = mybir.AluOpType
    TWO_PI = 2.0 * math.pi
    NF = b_core // P128
    assert Pv == 10.0, "d+5 sign trick assumes P == 10"

    delta = TWO_PI / Pv
    gamma = 9.5 * delta + math.pi / 2.0
    th_scale = sharpv / (2.0 * delta)
    th_bias = -gamma * th_scale

    nc = bacc.Bacc(
        "TRN2", target_bir_lowering=False, debug=False, num_devices=N_CORES
    )
    x_d = nc.dram_tensor("x_dram", [b_core, T], i32, kind="ExternalInput").ap()
    y_d = nc.dram_tensor("y_dram", [b_core, T], i32, kind="ExternalInput").ap()
    lg_d = nc.dram_tensor(
        "logits_dram", [b_core, T, D], fp32, kind="ExternalOutput"
    ).ap()
    cr_d = nc.dram_tensor("carry_dram", [b_core], fp32, kind="ExternalOutput").ap()

    xv = x_d.rearrange("(p n) t -> p (n t)", p=P128)
    yv = y_d.rearrange("(p n) t -> p (n t)", p=P128)
    lv = lg_d.rearrange("(p n) t d -> p n t d", p=P128)
    cv = cr_d.rearrange("(p n) -> p n", p=P128)

    nblk = T // tb
    seg = tb * NF  # elements per (d, t-block) segment

    with tile.TileContext(nc) as tc:
        with (
            tc.tile_pool(name="main", bufs=1) as mp,
            tc.tile_pool(name="stage", bufs=2) as stp,
            tc.tile_pool(name="hp", bufs=2) as hp,
            tc.tile_pool(name="sp", bufs=2) as sp,
            tc.tile_pool(name="lp", bufs=2) as lp,
        ):
            # ---- prep: zs = delta*(x+y) + (gamma - 9*delta), fp32, (n,t) ----
            zs = mp.tile([P128, NF * T], fp32, tag="zs")
            n_ch = min(8, NF)  # n-rows per staging chunk
            for c0 in range(0, NF, n_ch):
                sl = slice(c0 * T, (c0 + n_ch) * T)
                xch = stp.tile([P128, n_ch * T], i32, tag="xch")
                ych = stp.tile([P128, n_ch * T], i32, tag="ych")
                nc.sync.dma_start(xch[:], xv[:, sl])
                nc.sync.dma_start(ych[:], yv[:, sl])
                nc.vector.tensor_tensor(zs[:, sl], xch[:], ych[:], Alu.add)
                nc.vector.tensor_scalar(
                    zs[:, sl], zs[:, sl], float(delta),
                    float(gamma - 9.0 * delta), Alu.mult, Alu.add,
                )
            zs3 = zs[:].rearrange("p (n t) -> p n t", t=T)

            # ---- carry recurrence; Z is t-major so chain writes contiguous --
            Z = mp.tile([P128, NF * T], fp32, tag="Z")
            Zt = Z[:].rearrange("p (t n) -> p t n", n=NF)

            thb = mp.tile([P128, 1], fp32, tag="thb")
            nc.vector.memset(thb[:], float(th_bias))

            h_prev = hp.tile([P128, NF], fp32, tag="h")
            nc.vector.memset(h_prev[:], -1.0)
            for t in range(T):
                nc.vector.scalar_tensor_tensor(
                    Zt[:, t, :], h_prev[:], float(delta * 0.5),
                    zs3[:, :, t], Alu.mult, Alu.add,
                )
                h_new = hp.tile([P128, NF], fp32, tag="h")
                nc.scalar.activation(
                    h_new[:], Zt[:, t, :], A.Tanh, bias=thb[:],
                    scale=float(th_scale),
                )
                h_prev = h_new

            cfin = hp.tile([P128, NF], fp32, tag="cfin")
            nc.vector.tensor_scalar(cfin[:], h_prev[:], 0.5, 0.5, Alu.mult, Alu.add)
            nc.sync.dma_start(cv, cfin[:])

            # ---- base range reduction, in place: Z <- wrap(Z - 2pi) ---------
            nc.vector.add_range_wrap(
                Z[:], Z[:], float(-TWO_PI), float(math.pi), float(TWO_PI)
            )

            # ---- per t-block: wrap per d, sin, negate, scatter, DMA out -----
            for bi in range(nblk):
                b0 = bi * tb
                w0b = Z[:, b0 * NF : (b0 + tb) * NF]  # contiguous (t,n) slice
                S = sp.tile([P128, D * seg], fp32, tag="S")
                for d in range(5):
                    nc.vector.add_range_wrap(
                        S[:, d * seg : (d + 1) * seg], w0b,
                        float(-delta * d), float(math.pi), float(TWO_PI),
                    )
                # sin over the 5 lower segments, in place
                nc.scalar.activation(
                    S[:, 0 : 5 * seg], S[:, 0 : 5 * seg], A.Sin,
                    bias=0.0, scale=1.0,
                )
                # upper half = negated lower half (contiguous 1-src copy)
                nc.vector.tensor_scalar_mul(
                    S[:, 5 * seg : 10 * seg], S[:, 0 : 5 * seg], -1.0
                )
                # scatter: strided READ of S in (n,t,d) order, contiguous write
                Sv = S[:].rearrange("p (d t n) -> p n t d", d=D, t=tb)
                Lt = lp.tile([P128, NF * tb * D], fp32, tag="L")
                L4 = Lt[:].rearrange("p (n t d) -> p n t d", t=tb, d=D)
                nc.gpsimd.tensor_scalar_mul(L4, Sv, float(Lv))
                nc.sync.dma_start(lv[:, :, b0 : b0 + tb, :], L4)

    nc.compile()
    return nc


def kernel(x_digits_rev, y_digits_rev, P, sharp, logit_scale):
    from concourse import bass_utils

    x = np.ascontiguousarray(np.asarray(x_digits_rev), dtype=np.int32)
    y = np.ascontiguousarray(np.asarray(y_digits_rev), dtype=np.int32)
    Pv = float(np.asarray(P))
    sv = float(np.asarray(sharp))
    Lv = float(np.asarray(logit_scale))
    key = (Pv, sv, Lv)
    if key not in _cache:
        _cache[key] = build(Pv, sv, Lv)
    nc = _cache[key]
    in_maps = [
        {
            "x_dram": np.ascontiguousarray(x[c * B_CORE : (c + 1) * B_CORE]),
            "y_dram": np.ascontiguousarray(y[c * B_CORE : (c + 1) * B_CORE]),
        }
        for c in range(N_CORES)
    ]
    res = bass_utils.run_bass_kernel_spmd(nc, in_maps, core_ids=list(range(N_CORES)))
    logits = np.concatenate(
        [res.results[c]["logits_dram"] for c in range(N_CORES)], axis=0
    )
    carry = np.concatenate(
        [res.results[c]["carry_dram"] for c in range(N_CORES)], axis=0
    )
    return logits, carry


# revision 9
# speedup vs baseline: 3.5491x; 3.5491x over previous
import math
import sys

sys.path.insert(0, "/opt/trn_rl_repo")

import numpy as np

N_CORES = 8
B, T, D = 65536, 64, 10
B_CORE = B // N_CORES  # 8192
P128 = 128

_cache = {}


def build(Pv, sharpv, Lv, b_core=B_CORE, tb=16):
    """Build + compile the per-core SPMD Bass program.

    Math: s_t = x_t + y_t, carry c_t (c_0 = 0), u_t = s_t + c_t:
      c_{t+1} = sigmoid(sharp*(u_t - 9.5))
      logits[t,d] = L*cos((2pi/P)*(u_t - d))

    On-chip: h_t = tanh(sharp*(u_t-9.5)/2) = 2*c_{t+1}-1 (Tanh+Sin share an
    ACT table set; Sigmoid+Sin do not), angle tile Z = delta*u + pi/2 with
    delta = 2pi/P:
      Z_t = (delta/2)*h_{t-1} + [delta*(s_t - 9) + gamma]   (h_{-1} = -1)
      logits[t,d]   = L*sin(wrap(Z_t - delta*d))    d in [0,5)
      logits[t,d+5] = -logits[t,d]                  (5*delta = pi for P=10)

    Perf rules (from HW traces): compute-engine WRITES must be unit-stride
    (strided writes run ~14x slow); strided READS are ~free. So every op
    writes contiguous; the one reorder (d-major S -> d-minor L) happens in
    the scatter op via a strided READ view, and DMA out moves 640B runs.
    """
    import concourse.bacc as bacc
    import concourse.mybir as mybir
    import concourse.tile as tile

    fp32 = mybir.dt.float32
    i32 = mybir.dt.int32
    A = mybir.ActivationFunctionType
    Alu = mybir.AluOpType
    TWO_PI = 2.0 * math.pi
    NF = b_core // P128
    assert Pv == 10.0, "d+5 sign trick assumes P == 10"

    delta = TWO_PI / Pv
    gamma = 9.5 * delta + math.pi / 2.0
    th_scale = sharpv / (2.0 * delta)
    th_bias = -gamma * th_scale

    nc = bacc.Bacc(
        "TRN2", target_bir_lowering=False, debug=False, num_devices=N_CORES
    )
    x_d = nc.dram_tensor("x_dram", [b_core, T], i32, kind="ExternalInput").ap()
    y_d = nc.dram_tensor("y_dram", [b_core, T], i32, kind="ExternalInput").ap()
    lg_d = nc.dram_tensor(
        "logits_dram", [b_core, T, D], fp32, kind="ExternalOutput"
    ).ap()
    cr_d = nc.dram_tensor("carry_dram", [b_core], fp32, kind="ExternalOutput").ap()

    xv = x_d.rearrange("(p n) t -> p (n t)", p=P128)
    yv = y_d.rearrange("(p n) t -> p (n t)", p=P128)
    lv = lg_d.rearrange("(p n) t d -> p n t d", p=P128)
    cv = cr_d.rearrange("(p n) -> p n", p=P128)

    nblk = T // tb
    seg = tb * NF  # elements per (d, t-block) segment

    with tile.TileContext(nc) as tc:
        with (
            tc.tile_pool(name="main", bufs=1) as mp,
            tc.tile_pool(name="stage", bufs=2) as stp,
            tc.tile_pool(name="hp", bufs=2) as hp,
            tc.tile_pool(name="sp", bufs=2) as sp,
            tc.tile_pool(name="lp", bufs=2) as lp,
        ):
            # ---- prep: zs = delta*(x+y) + (gamma - 9*delta), fp32, (n,t) ----
            zs = mp.tile([P128, NF * T], fp32, tag="zs")
            n_ch = min(8, NF)  # n-rows per staging chunk
            for c0 in range(0, NF, n_ch):
                sl = slice(c0 * T, (c0 + n_ch) * T)
                xch = stp.tile([P128, n_ch * T], i32, tag="xch")
                ych = stp.tile([P128, n_ch * T], i32, tag="ych")
                nc.sync.dma_start(xch[:], xv[:, sl])
                nc.sync.dma_start(ych[:], yv[:, sl])
                nc.vector.tensor_tensor(zs[:, sl], xch[:], ych[:], Alu.add)
                nc.vector.tensor_scalar(
                    zs[:, sl], zs[:, sl], float(delta),
                    float(gamma - 9.0 * delta), Alu.mult, Alu.add,
                )
            zs3 = zs[:].rearrange("p (n t) -> p n t", t=T)

            # ---- carry recurrence; Z is t-major so chain writes contiguous --
            Z = mp.tile([P128, NF * T], fp32, tag="Z")
            Zt = Z[:].rearrange("p (t n) -> p t n", n=NF)

            thb = mp.tile([P128, 1], fp32, tag="thb")
            nc.vector.memset(thb[:], float(th_bias))

            h_prev = hp.tile([P128, NF], fp32, tag="h")
            nc.vector.memset(h_prev[:], -1.0)
            for t in range(T):
                nc.vector.scalar_tensor_tensor(
                    Zt[:, t, :], h_prev[:], float(delta * 0.5),
                    zs3[:, :, t], Alu.mult, Alu.add,
                )
                h_new = hp.tile([P128, NF], fp32, tag="h")
                nc.scalar.activation(
                    h_new[:], Zt[:, t, :], A.Tanh, bias=thb[:],
                    scale=float(th_scale),
                )
                h_prev = h_new

            cfin = hp.tile([P128, NF], fp32, tag="cfin")
            nc.vector.tensor_scalar(cfin[:], h_prev[:], 0.5, 0.5, Alu.mult, Alu.add)
            nc.sync.dma_start(cv, cfin[:])

            # ---- base range reduction, in place: Z <- wrap(Z - 2pi) ---------
            nc.vector.add_range_wrap(
                Z[:], Z[:], float(-TWO_PI), float(math.pi), float(TWO_PI)
            )

            # ---- per t-block: wrap per d, sin, negate, scatter, DMA out -----
            for bi in range(nblk):
                b0 = bi * tb
                w0b = Z[:, b0 * NF : (b0 + tb) * NF]  # contiguous (t,n) slice
                S = sp.tile([P128, D * seg], fp32, tag="S")
                for d in range(5):
                    nc.vector.add_range_wrap(
                        S[:, d * seg : (d + 1) * seg], w0b,
                        float(-delta * d), float(math.pi), float(TWO_PI),
                    )
                # sin over the 5 lower segments, in place
                nc.scalar.activation(
                    S[:, 0 : 5 * seg], S[:, 0 : 5 * seg], A.Sin,
                    bias=0.0, scale=1.0,
                )
                # upper half = negated lower half (contiguous 1-src copy)
                # NB: op1 must not be `bypass` — MULTIPLY,BYPASS runs ~14
                # cycles/elem on HW; MULTIPLY,ADD streams at ~1/cycle.
                nc.vector.tensor_scalar(
                    S[:, 5 * seg : 10 * seg], S[:, 0 : 5 * seg], -1.0, 0.0,
                    Alu.mult, Alu.add,
                )
                # scatter: strided READ of S in (n,t,d) order, contiguous write
                Sv = S[:].rearrange("p (d t n) -> p n t d", d=D, t=tb)
                Lt = lp.tile([P128, NF * tb * D], fp32, tag="L")
                L4 = Lt[:].rearrange("p (n t d) -> p n t d", t=tb, d=D)
                nc.gpsimd.tensor_scalar(L4, Sv, float(Lv), 0.0, Alu.mult, Alu.add)
                nc.sync.dma_start(lv[:, :, b0 : b0 + tb, :], L4)

    nc.compile()
    return nc


def kernel(x_digits_rev, y_digits_rev, P, sharp, logit_scale):
    from concourse import bass_utils

    x = np.ascontiguousarray(np.asarray(x_digits_rev), dtype=np.int32)
    y = np.ascontiguousarray(np.asarray(y_digits_rev), dtype=np.int32)
    Pv = float(np.asarray(P))
    sv = float(np.asarray(sharp))
    Lv = float(np.asarray(logit_scale))
    key = (Pv, sv, Lv)
    if key not in _cache:
        _cache[key] = build(Pv, sv, Lv)
    nc = _cache[key]
    in_maps = [
        {
            "x_dram": np.ascontiguousarray(x[c * B_CORE : (c + 1) * B_CORE]),
            "y_dram": np.ascontiguousarray(y[c * B_CORE : (c + 1) * B_CORE]),
        }
        for c in range(N_CORES)
    ]
    res = bass_utils.run_bass_kernel_spmd(nc, in_maps, core_ids=list(range(N_CORES)))
    logits = np.concatenate(
        [res.results[c]["logits_dram"] for c in range(N_CORES)], axis=0
    )
    carry = np.concatenate(
        [res.results[c]["carry_dram"] for c in range(N_CORES)], axis=0
    )
    return logits, carry
